# revision 19
# baseline (speedup 1.0000x reference)
# Trainium2 Bass kernel for NeuroSSMLayer.
#
# Sharding: data-parallel over batch (B=8) across 8 NeuronCores; weights
# replicated. Each core computes one batch element end-to-end; the time
# scan is local per core.
#
# Math (per batch b):
#   xB = x @ Bm.T            [T,S]
#   s_t = tanh(A s_{t-1} + xB_t)          (sequential scan)
#   out = S @ C.T + x @ D.T  [T,H]
#   y   = LN(out + x)
#   y2  = y + W2 @ gelu(W1 @ y + b1) + b2
# returns (y2 [B,T,H], final_state [B,S])
#
# The scan is parallelized over time chunks: chunk c starts from a state
# approximated by running WARM steps of the recurrence from zero state
# (tanh saturation contracts ~0.55x/step; WARM=32 gives ~2e-6 state err).
# Chunk 0 uses zero-padded inputs so it is exact.
#
# All big GEMMs run in fp16 (1 cyc/row on PE) with fp32 PSUM accumulation;
# LayerNorm/residuals in fp32. Validated end-to-end rel err ~4e-4.

import numpy as np

T = 2048
H = 1024
S = 256
F = 4096
B = 8
LN_EPS = 1e-5

CHUNK = 32          # scan chunk length
WARM = 32           # scan warmup steps
NCH = T // CHUNK    # 64 chunks
TBLK = 256          # phase-B time block

_cached = {}


def _build(phases="ABT"):
    import concourse.bass as bass
    import concourse.mybir as mybir
    from concourse import bacc
    from concourse.tile import TileContext
    from concourse.masks import make_identity
    from contextlib import ExitStack

    f32 = mybir.dt.float32
    f16 = mybir.dt.float16

    nc = bacc.Bacc()

    xb = nc.dram_tensor("xb", [T, H], f32, kind="ExternalInput")
    A_in = nc.dram_tensor("A_in", [S, S], f32, kind="ExternalInput")
    Bm_in = nc.dram_tensor("Bm_in", [S, H], f32, kind="ExternalInput")
    C_in = nc.dram_tensor("C_in", [H, S], f32, kind="ExternalInput")
    D_in = nc.dram_tensor("D_in", [H, H], f32, kind="ExternalInput")
    lnw_in = nc.dram_tensor("lnw_in", [H], f32, kind="ExternalInput")
    lnb_in = nc.dram_tensor("lnb_in", [H], f32, kind="ExternalInput")
    W1_in = nc.dram_tensor("W1_in", [F, H], f32, kind="ExternalInput")
    b1_in = nc.dram_tensor("b1_in", [F], f32, kind="ExternalInput")
    W2_in = nc.dram_tensor("W2_in", [H, F], f32, kind="ExternalInput")
    b2_in = nc.dram_tensor("b2_in", [H], f32, kind="ExternalInput")

    y_out = nc.dram_tensor("y_out", [T, H], f32, kind="ExternalOutput")
    fs_out = nc.dram_tensor("fs_out", [S], f32, kind="ExternalOutput")

    # fp16 staging copies in DRAM (cast during DMA on SWDGE), consumed by
    # hardware DMA-transpose loads.
    xh = nc.dram_tensor("xh", [T, H], f16)
    ah = nc.dram_tensor("ah", [S, S], f16)
    bmh = nc.dram_tensor("bmh", [S, H], f16)
    ch = nc.dram_tensor("ch", [H, S], f16)
    dh = nc.dram_tensor("dh", [H, H], f16)
    w1h = nc.dram_tensor("w1h", [F, H], f16)
    w2h = nc.dram_tensor("w2h", [H, F], f16)
    yth = nc.dram_tensor("yth", [T, H], f16)

    with TileContext(nc) as tc, ExitStack() as top:
        # ---- phase 0: cast everything we need as fp16 to DRAM ----
        for dst, src in ((xh, xb), (ah, A_in), (bmh, Bm_in), (ch, C_in),
                         (dh, D_in), (w1h, W1_in), (w2h, W2_in)):
            nc.gpsimd.dma_start(out=dst[:, :], in_=src[:, :])

        singles = top.enter_context(tc.tile_pool(name="singles", bufs=1))
        ident32 = singles.tile([128, 128], f32)
        make_identity(nc, ident32)
        ident16 = singles.tile([128, 128], f16)
        nc.vector.tensor_copy(ident16, ident32)
        eps_t = singles.tile([128, 1], f32)
        nc.vector.memset(eps_t, LN_EPS)

        # broadcast LN affine params across partitions
        w_bc = singles.tile([128, H], f32)
        nc.gpsimd.dma_start(
            out=w_bc,
            in_=bass.AP(tensor=lnw_in[:].tensor, offset=0, ap=[[0, 128], [1, H]]),
        )
        b_bc = singles.tile([128, H], f32)
        nc.gpsimd.dma_start(
            out=b_bc,
            in_=bass.AP(tensor=lnb_in[:].tensor, offset=0, ap=[[0, 128], [1, H]]),
        )

        # =================== PHASE A ===================
        with ExitStack() as pa:
            pa_sing = pa.enter_context(tc.tile_pool(name="pa_sing", bufs=1))

            # transposed fp16 operands via xbar DMA-transpose
            at_t = pa_sing.tile([128, 2, S], f16)       # A.T   [s_in, s_out]
            for j in range(2):
                nc.sync.dma_start_transpose(at_t[:, j, :], ah[:, j * 128:(j + 1) * 128])
            bmt = pa_sing.tile([128, 8, S], f16)        # Bm.T  [h, s]
            for j in range(8):
                nc.sync.dma_start_transpose(bmt[:, j, :], bmh[:, j * 128:(j + 1) * 128])
            cdt = pa_sing.tile([128, 10, H], f16)       # [C.T ; D.T]  [(s|h), h_out]
            for j in range(2):
                nc.sync.dma_start_transpose(cdt[:, j, :], ch[:, j * 128:(j + 1) * 128])
            for j in range(8):
                nc.sync.dma_start_transpose(cdt[:, 2 + j, :], dh[:, j * 128:(j + 1) * 128])
            xt = pa_sing.tile([128, 8, T], f16)         # x.T   [h, t]
            for j in range(8):
                nc.sync.dma_start_transpose(xt[:, j, :], xh[:, j * 128:(j + 1) * 128])

            # ---- xB = Bm @ x.T  -> padded fp16 [s, WARM+T] ----
            xbp = pa_sing.tile([128, 2, WARM + T], f16)
            nc.vector.memset(xbp[:, :, 0:WARM], 0.0)
            xb_ps = pa.enter_context(tc.tile_pool(name="xb_ps", bufs=2, space="PSUM"))
            for tt in range(T // 512):
                tsl = slice(tt * 512, (tt + 1) * 512)
                for m in range(2):
                    ps = xb_ps.tile([128, 512], f32, tag="xbps")
                    for k in range(8):
                        nc.tensor.matmul(
                            ps, bmt[:, k, m * 128:(m + 1) * 128], xt[:, k, tsl],
                            start=(k == 0), stop=(k == 7),
                        )
                    nc.scalar.copy(xbp[:, m, WARM + tt * 512: WARM + (tt + 1) * 512], ps)

            # ---- chunked scan ----
            st_a = pa_sing.tile([128, 2 * NCH], f16)
            st_b = pa_sing.tile([128, 2 * NCH], f16)
            nc.vector.memset(st_a, 0.0)
            sseq = pa_sing.tile([128, 2, T], f16)
            fs32 = pa_sing.tile([128, 2], f32)
            scan_ps = pa.enter_context(tc.tile_pool(name="scan_ps", bufs=2, space="PSUM"))
            cur, nxt = st_a, st_b
            for i in range(WARM + CHUNK):
                ps = scan_ps.tile([128, 2 * NCH], f32, tag="scanps")
                for m in range(2):
                    msl = slice(m * NCH, (m + 1) * NCH)
                    u = xbp[:, m, i::CHUNK][:, :NCH]
                    nc.tensor.matmul(ps[:, msl], ident16, u, start=True, stop=False)
                    for k in range(2):
                        nc.tensor.matmul(
                            ps[:, msl],
                            at_t[:, k, m * 128:(m + 1) * 128],
                            cur[:, k * NCH:(k + 1) * NCH],
                            start=False, stop=(k == 1),
                        )
                nc.scalar.activation(nxt, ps, mybir.ActivationFunctionType.Tanh)
                if i >= WARM:
                    nc.vector.tensor_copy(
                        sseq[:, :, (i - WARM)::CHUNK][:, :, :NCH],
                        nxt.rearrange("p (j n) -> p j n", j=2),
                    )
                if i == WARM + CHUNK - 1:
                    nc.scalar.activation(
                        fs32, ps[:, NCH - 1::NCH], mybir.ActivationFunctionType.Tanh
                    )
                cur, nxt = nxt, cur
            nc.sync.dma_start(
                out=fs_out[:].rearrange("(j p) -> p j", j=2), in_=fs32
            )

            # ---- out = [S ; x] @ [C ; D].T, residual, LayerNorm ----
            cd_ps = pa.enter_context(tc.tile_pool(name="cd_ps", bufs=2, space="PSUM"))
            ln_pool = pa.enter_context(tc.tile_pool(name="ln", bufs=3))
            for tt in range(T // 128):
                tsl = slice(tt * 128, (tt + 1) * 128)
                x_sb = ln_pool.tile([128, H], f32, tag="xres")
                nc.sync.dma_start(out=x_sb, in_=xb[tsl, :])
                z = ln_pool.tile([128, H], f32, tag="z")
                for nh in range(2):
                    nsl = slice(nh * 512, (nh + 1) * 512)
                    ps = cd_ps.tile([128, 512], f32, tag="cdps")
                    for k in range(10):
                        lhsT = (sseq[:, k, tsl] if k < 2 else xt[:, k - 2, tsl])
                        nc.tensor.matmul(ps, lhsT, cdt[:, k, nsl],
                                         start=(k == 0), stop=(k == 9))
                    nc.vector.tensor_add(z[:, nsl], ps, x_sb[:, nsl])
                stats = ln_pool.tile([128, 2, 6], f32, tag="stats")
                for g in range(2):
                    nc.vector.bn_stats(stats[:, g, :], z[:, g * 512:(g + 1) * 512])
                mv = ln_pool.tile([128, 2], f32, tag="mv")
                nc.vector.bn_aggr(mv, stats)
                rstd = ln_pool.tile([128, 1], f32, tag="rstd")
                nc.scalar.activation(
                    rstd, mv[:, 1:2], mybir.ActivationFunctionType.Sqrt, bias=eps_t
                )
                nc.vector.reciprocal(rstd, rstd)
                y0 = ln_pool.tile([128, H], f32, tag="y0")
                nc.vector.tensor_scalar(
                    out=y0, in0=z, scalar1=mv[:, 0:1], scalar2=rstd,
                    op0=mybir.AluOpType.subtract, op1=mybir.AluOpType.mult,
                )
                y1 = ln_pool.tile([128, H], f32, tag="y1")
                nc.vector.tensor_mul(y1, y0, w_bc)
                y16 = ln_pool.tile([128, H], f16, tag="y16")
                nc.vector.tensor_add(y16, y1, b_bc)
                nc.sync.dma_start(out=yth[tsl, :], in_=y16)

        # =================== PHASE B ===================
        if "B" in phases:
            with ExitStack() as pb:
                pb_sing = pb.enter_context(tc.tile_pool(name="pb_sing", bufs=1))
                w1t = pb_sing.tile([128, 8, F], f16)        # W1.T [h, f]
                for j in range(8):
                    nc.sync.dma_start_transpose(w1t[:, j, :], w1h[:, j * 128:(j + 1) * 128])
                w2t = pb_sing.tile([128, 32, H], f16)       # W2.T [f, h]
                for j in range(32):
                    nc.sync.dma_start_transpose(w2t[:, j, :], w2h[:, j * 128:(j + 1) * 128])
                b1_sb = pb_sing.tile([128, 32], f32)
                nc.sync.dma_start(out=b1_sb, in_=b1_in[:].rearrange("(j p) -> p j", j=32))
                b2_sb = pb_sing.tile([128, 8], f32)
                nc.sync.dma_start(out=b2_sb, in_=b2_in[:].rearrange("(j p) -> p j", j=8))

                yt_pool = pb.enter_context(tc.tile_pool(name="yt", bufs=2))
                h1_pool = pb.enter_context(tc.tile_pool(name="h1", bufs=2))
                o_pool = pb.enter_context(tc.tile_pool(name="o", bufs=2))
                f1_ps = pb.enter_context(tc.tile_pool(name="f1ps", bufs=4, space="PSUM"))
                f2_ps = pb.enter_context(tc.tile_pool(name="f2ps", bufs=2, space="PSUM"))
                tp_ps = pb.enter_context(tc.tile_pool(name="tpps", bufs=2, space="PSUM"))

                for blk in range(T // TBLK):
                    bsl = slice(blk * TBLK, (blk + 1) * TBLK)
                    ytb = yt_pool.tile([128, 8, TBLK], f16, tag="ytb")
                    for j in range(8):
                        nc.sync.dma_start_transpose(
                            ytb[:, j, :], yth[bsl, j * 128:(j + 1) * 128]
                        )
                    h1 = h1_pool.tile([128, 32, TBLK], f16, tag="h1")
                    for f in range(32):
                        ps = f1_ps.tile([128, TBLK], f32, tag="f1")
                        for k in range(8):
                            nc.tensor.matmul(
                                ps, w1t[:, k, f * 128:(f + 1) * 128], ytb[:, k, :],
                                start=(k == 0), stop=(k == 7),
                            )
                        nc.scalar.activation(
                            h1[:, f, :], ps, mybir.ActivationFunctionType.Gelu,
                            bias=b1_sb[:, f:f + 1],
                        )
                    y2t = o_pool.tile([128, 8, TBLK], f16, tag="y2t")
                    for hh in range(8):
                        ps = f2_ps.tile([128, TBLK], f32, tag="f2")
                        nc.tensor.matmul(ps, ident16, ytb[:, hh, :], start=True, stop=False)
                        for k in range(32):
                            nc.tensor.matmul(
                                ps, w2t[:, k, hh * 128:(hh + 1) * 128], h1[:, k, :],
                                start=False, stop=(k == 31),
                            )
                        nc.vector.tensor_scalar_add(y2t[:, hh, :], ps, b2_sb[:, hh:hh + 1])
                    if "T" in phases:
                        for tt in range(TBLK // 128):
                            y2 = o_pool.tile([128, H], f32, tag="y2")
                            for hh in range(8):
                                ps = tp_ps.tile([128, 128], f16, tag="tp")
                                nc.tensor.transpose(
                                    ps, y2t[:, hh, tt * 128:(tt + 1) * 128], ident16
                                )
                                nc.scalar.copy(y2[:, hh * 128:(hh + 1) * 128], ps)
                            nc.sync.dma_start(
                                out=y_out[blk * TBLK + tt * 128: blk * TBLK + (tt + 1) * 128, :],
                                in_=y2,
                            )
                    else:
                        # crash-bisect variant: scattered store, correct values
                        for j in range(8):
                            nc.sync.dma_start(
                                out=y_out[bsl, j * 128:(j + 1) * 128].rearrange("t p -> p t"),
                                in_=y2t[:, j, :],
                            )
    nc.finalize()
    return nc


def _get_nc():
    if "nc" not in _cached:
        _cached["nc"] = _build()
    return _cached["nc"]


def _stub_axon_hooks():
    # bass_utils' BASS_TRACE path imports antenv.axon_hooks, which does not
    # exist in this environment; stub it so trace requests degrade gracefully.
    import sys, types
    if "antenv.axon_hooks" not in sys.modules:
        try:
            import antenv.axon_hooks  # noqa: F401
        except ImportError:
            m = types.ModuleType("antenv.axon_hooks")
            m.get_axon_ntff_profile_hook = lambda: None
            sys.modules["antenv.axon_hooks"] = m


def kernel(x, A, Bm, C, D, ln_w, ln_b, W1, b1, W2, b2):
    from concourse.bass_utils import run_bass_kernel_spmd
    _stub_axon_hooks()

    x = np.ascontiguousarray(np.asarray(x, dtype=np.float32))
    weights = {
        "A_in": A, "Bm_in": Bm, "C_in": C, "D_in": D,
        "lnw_in": ln_w, "lnb_in": ln_b,
        "W1_in": W1, "b1_in": b1, "W2_in": W2, "b2_in": b2,
    }
    weights = {k: np.ascontiguousarray(np.asarray(v, dtype=np.float32))
               for k, v in weights.items()}

    nc = _get_nc()
    in_maps = [dict(weights, xb=np.ascontiguousarray(x[b])) for b in range(B)]
    res = run_bass_kernel_spmd(nc, in_maps, core_ids=list(range(B)))
    _cached["last_res"] = res
    y = np.stack([r["y_out"] for r in res.results], axis=0)
    fs = np.stack([r["fs_out"] for r in res.results], axis=0)
    return (y, fs)
